# revision 20
# baseline (speedup 1.0000x reference)
"""CPM3 attention kernel for 8 trn2 NeuronCores.

Sharding: tensor-parallel over heads (2 heads/core x both batches).
Device computes per-core partial outputs (Wo row-sharded); host sums.
"""

import sys

sys.path.insert(0, "/opt/trn_rl_repo")

import numpy as np
import ml_dtypes

import concourse.bass as bass
import concourse.bacc as bacc
import concourse.tile as tile
import concourse.mybir as mybir
from concourse.bass_utils import run_bass_kernel_spmd

B, L, D, H, DH = 2, 2048, 1024, 16, 64
N_CORES = 8
HPC = H // N_CORES  # heads per core = 2
D2H = HPC * DH  # 128
QTS = 512  # q tile size
QN = L // QTS  # 4
KP = 128  # k partition tile
KN = L // KP  # 16
KTG = 4  # k tiles per DMA group
KGN = KN // KTG  # 4
DC = D // 128  # 8 contraction chunks
HVW = 2 * (DH + 1)  # 130: hv_aug columns per k-tile (2 heads x (64+ones))

F32 = mybir.dt.float32
F32R = mybir.dt.float32r
U8 = mybir.dt.uint8
BF16 = mybir.dt.bfloat16
MM_NEG = 1.0e9  # additive mask magnitude

_CACHE: dict = {}


def _build():
    if "nc" in _CACHE:
        return _CACHE["nc"]
    nc = bacc.Bacc("TRN2", target_bir_lowering=False, debug=False, num_devices=N_CORES)

    qT = nc.dram_tensor("qT", [B, DC, 128, L], F32R, kind="ExternalInput").ap()
    kvT = nc.dram_tensor("kvT", [B, DC, 128, L], F32R, kind="ExternalInput").ap()
    wq = nc.dram_tensor("wq", [128, DC, 128], F32R, kind="ExternalInput").ap()
    wk = nc.dram_tensor("wk", [128, DC, 128], F32R, kind="ExternalInput").ap()
    wv = nc.dram_tensor("wv", [128, DC, 128], F32R, kind="ExternalInput").ap()
    wo = nc.dram_tensor("wo", [128, D], F32R, kind="ExternalInput").ap()
    pb = nc.dram_tensor(
        "pb", [QN, KGN, 128, KTG, HPC, QTS], F32R, kind="ExternalInput"
    ).ap()
    mk = nc.dram_tensor("mk", [QN, 128, B, KN, QTS], U8, kind="ExternalInput").ap()
    ident = nc.dram_tensor("ident", [128, 128], F32R, kind="ExternalInput").ap()
    indh = nc.dram_tensor("indh", [1, 256], F32R, kind="ExternalInput").ap()
    out = nc.dram_tensor("out", [B, L, D], F32, kind="ExternalOutput").ap()

    with tile.TileContext(nc) as tc:
        with (
            tc.tile_pool(name="const", bufs=1) as constp,
            tc.tile_pool(name="hq", bufs=2) as hqp,
            tc.tile_pool(name="hk", bufs=2) as hkp,
            tc.tile_pool(name="hv", bufs=2) as hvp,
            tc.tile_pool(name="stage", bufs=2) as stagep,
            tc.tile_pool(name="pbp", bufs=2) as pbp,
            tc.tile_pool(name="mask", bufs=2) as mkp,
            tc.tile_pool(name="ma", bufs=8) as map_,
            tc.tile_pool(name="pt", bufs=6) as ptp,
            tc.tile_pool(name="ctxn", bufs=2) as ctxnp,
            tc.tile_pool(name="rc", bufs=2) as rcp,
            tc.tile_pool(name="outb", bufs=3) as outp,
            tc.tile_pool(name="psum", bufs=8, space=bass.MemorySpace.PSUM) as psp,
        ):
            # ---- constants ----
            ident_t = constp.tile([128, 128], F32R, tag="ident")
            nc.sync.dma_start(ident_t[:], ident[:])
            indh_t = constp.tile([1, 256], F32R, tag="indh")
            nc.sync.dma_start(indh_t[:], indh[:])
            wq_t = constp.tile([128, DC, 128], F32R, tag="wq")
            nc.sync.dma_start(wq_t[:], wq[:])
            wk_t = constp.tile([128, DC, 128], F32R, tag="wk")
            nc.sync.dma_start(wk_t[:], wk[:])
            wv_t = constp.tile([128, DC, 128], F32R, tag="wv")
            nc.sync.dma_start(wv_t[:], wv[:])
            wo_t = constp.tile([128, D], F32R, tag="wo")
            nc.sync.dma_start(wo_t[:], wo[:])

            # ---- prologue: projections ----
            hq_sb, hk_sb, hv_sb = {}, {}, {}
            for b in range(B):
                hq_sb[b] = hqp.tile([128, L], F32R, tag="hq", name=f"hq_sb{b}")
                hq_ps = [psp.tile([128, QTS], F32, tag="bank", name=f"hq_ps{b}_{i}") for i in range(QN)]
                for dc in range(DC):
                    qc = stagep.tile([128, L], F32R, tag="stage")
                    nc.sync.dma_start(qc[:], qT[b, dc])
                    for qt in range(QN):
                        nc.tensor.matmul(
                            hq_ps[qt][:],
                            wq_t[:, dc, :],
                            qc[:, qt * QTS : (qt + 1) * QTS],
                            start=(dc == 0),
                            stop=(dc == DC - 1),
                        )
                for qt in range(QN):
                    nc.vector.tensor_copy(
                        hq_sb[b][:, qt * QTS : (qt + 1) * QTS],
                        hq_ps[qt][:],
                    )

                hk_sb[b] = hkp.tile([128, L], F32R, tag="hk", name=f"hk_sb{b}")
                hvT = stagep.tile([128, L], F32R, tag="stage")
                hk_ps = [psp.tile([128, QTS], F32, tag="bank", name=f"hk_ps{b}_{i}") for i in range(QN)]
                hv_ps = [psp.tile([128, QTS], F32, tag="bank", name=f"hv_ps{b}_{i}") for i in range(QN)]
                for dc in range(DC):
                    kc = stagep.tile([128, L], F32R, tag="stage")
                    nc.sync.dma_start(kc[:], kvT[b, dc])
                    for qt in range(QN):
                        nc.tensor.matmul(
                            hk_ps[qt][:],
                            wk_t[:, dc, :],
                            kc[:, qt * QTS : (qt + 1) * QTS],
                            start=(dc == 0),
                            stop=(dc == DC - 1),
                        )
                        nc.tensor.matmul(
                            hv_ps[qt][:],
                            wv_t[:, dc, :],
                            kc[:, qt * QTS : (qt + 1) * QTS],
                            start=(dc == 0),
                            stop=(dc == DC - 1),
                        )
                for qt in range(QN):
                    nc.vector.tensor_copy(
                        hk_sb[b][:, qt * QTS : (qt + 1) * QTS],
                        hk_ps[qt][:],
                    )
                    nc.vector.tensor_copy(
                        hvT[:, qt * QTS : (qt + 1) * QTS], hv_ps[qt][:]
                    )

                # hv_aug: transpose hvT per k-tile; ones cols prefilled
                hv_sb[b] = hvp.tile([128, KN * HVW], F32R, tag="hv", name=f"hv_sb{b}")
                nc.gpsimd.memset(hv_sb[b][:].bitcast(mybir.dt.uint32), 0x3F800000)
                for kt in range(KN):
                    tp = psp.tile([128, 128], F32R, tag="bank")
                    nc.tensor.transpose(
                        tp[:], hvT[:, kt * KP : (kt + 1) * KP], ident_t[:]
                    )
                    o = kt * HVW
                    nc.vector.tensor_copy(hv_sb[b][:, o : o + DH], tp[:, 0:DH])
                    nc.vector.tensor_copy(
                        hv_sb[b][:, o + DH + 1 : o + 2 * DH + 1], tp[:, DH:128]
                    )

            # ---- main loop ----
            for qt in range(QN):
                mk_t = {}
                for b in range(B):
                    mk_t[b] = mkp.tile(
                        [128, KN, QTS], U8, tag="mask", name=f"mk_t{b}_{qt}"
                    )
                    nc.scalar.dma_start(mk_t[b][:], mk[qt, :, b])
                ctx_ps = {}
                for b in range(B):
                    for h in range(HPC):
                        ctx_ps[(b, h)] = psp.tile(
                            [DH + 1, QTS], F32, tag="bank", name=f"ctx_ps{b}_{h}_{qt}"
                        )
                for kg in range(KGN):
                    pb_t = pbp.tile([128, KTG, HPC, QTS], F32R, tag="pb")
                    nc.scalar.dma_start(pb_t[:], pb[qt, kg])
                    for ki in range(KTG):
                        kt = kg * KTG + ki
                        for b in range(B):
                            for h in range(HPC):
                                cmb = map_.tile(
                                    [128, QTS], F32R, tag="ma", name=f"cmb{b}_{h}_{kt}"
                                )
                                nc.vector.scalar_tensor_tensor(
                                    cmb[:],
                                    mk_t[b][:, kt, :],
                                    -MM_NEG,
                                    pb_t[:, ki, h, :],
                                    mybir.AluOpType.mult,
                                    mybir.AluOpType.add,
                                )
                                sc = psp.tile(
                                    [128, QTS], F32, tag="bank", name=f"sc{b}_{h}_{kt}"
                                )
                                nc.tensor.matmul(
                                    sc[:], ident_t[:], cmb[:], start=True, stop=False
                                )
                                nc.tensor.matmul(
                                    sc[:],
                                    hk_sb[b][h * DH : (h + 1) * DH, kt * KP : (kt + 1) * KP],
                                    hq_sb[b][h * DH : (h + 1) * DH, qt * QTS : (qt + 1) * QTS],
                                    start=False,
                                    stop=True,
                                )
                                p_t = ptp.tile(
                                    [128, QTS], F32R, tag="pt", name=f"p_t{b}_{h}_{kt}"
                                )
                                nc.scalar.activation(
                                    p_t[:], sc[:], mybir.ActivationFunctionType.Exp
                                )
                                o = kt * HVW + h * (DH + 1)
                                nc.tensor.matmul(
                                    ctx_ps[(b, h)][:],
                                    hv_sb[b][:, o : o + DH + 1],
                                    p_t[:],
                                    start=(kt == 0),
                                    stop=(kt == KN - 1),
                                )
                # normalize + output projection
                for b in range(B):
                    ctxn = ctxnp.tile([128, QTS], F32R, tag="ctxn")
                    bcw = psp.tile([128, QTS], F32, tag="bank", name=f"bcw{b}")
                    bc = bcw[:]
                    for h in range(HPC):
                        dsb = rcp.tile([1, QTS], F32, tag="dsb", name=f"dsb{b}_{h}")
                        nc.vector.tensor_copy(dsb[:], ctx_ps[(b, h)][DH : DH + 1, :])
                        rcf = rcp.tile([1, QTS], F32, tag="rcf", name=f"rcf{b}_{h}")
                        nc.vector.reciprocal_approx_fast(rcf[:], dsb[:])
                        rcr = rcp.tile([1, QTS], F32R, tag="rcr", name=f"rcr{b}_{h}")
                        nc.vector.tensor_copy(rcr[:], rcf[:])
                        nc.tensor.matmul(
                            bc,
                            indh_t[:, h * 128 : (h + 1) * 128],
                            rcr[:],
                            start=(h == 0),
                            stop=(h == HPC - 1),
                        )
                    bc_sb = rcp.tile([128, QTS], F32, tag="bcsb", name=f"bc_sb{b}")
                    nc.vector.tensor_copy(bc_sb[:], bc)
                    for h in range(HPC):
                        nc.vector.tensor_tensor(
                            ctxn[h * DH : (h + 1) * DH, :],
                            ctx_ps[(b, h)][0:DH, :],
                            bc_sb[h * DH : (h + 1) * DH, :],
                            mybir.AluOpType.mult,
                        )
                    for qs in range(QN):
                        ob = outp.tile([128, D], F32, tag="outb")
                        for oh in range(2):
                            op_ps = psp.tile(
                                [128, QTS], F32, tag="bank", name=f"op{b}_{qs}_{oh}"
                            )
                            nc.tensor.matmul(
                                op_ps[:],
                                ctxn[:, qs * 128 : (qs + 1) * 128],
                                wo_t[:, oh * QTS : (oh + 1) * QTS],
                                start=True,
                                stop=True,
                            )
                            nc.scalar.copy(
                                ob[:, oh * QTS : (oh + 1) * QTS], op_ps[:]
                            )
                        r0 = qt * QTS + qs * 128
                        nc.sync.dma_start(out[b, r0 : r0 + 128, :], ob[:])

    nc.compile()
    _CACHE["nc"] = nc
    return nc


def _prep_core(core, query, key_value, mask, position_bias, Wq, Wk, Wv, Wo, shared):
    """Per-core input map. `shared` holds core-independent packed arrays."""
    h0 = core * HPC
    rows = slice(h0 * DH, (h0 + HPC) * DH)
    # weight chunks packed [dp, dc, d2h] from W[rows].T [D, 128]
    def packw(w, scale=1.0):
        wt = np.ascontiguousarray(
            (w[rows].T * scale).reshape(DC, 128, 128).transpose(1, 0, 2),
            dtype=np.float32,
        )
        return wt

    pbc = position_bias[h0 : h0 + HPC]  # [2, q, k]
    # -> [qt, kg, kp, ki, h, qf]
    pbp = np.ascontiguousarray(
        pbc.reshape(HPC, QN, QTS, KGN, KTG, 128).transpose(1, 3, 5, 4, 0, 2),
        dtype=np.float32,
    )
    m = {
        "qT": shared["qT"],
        "kvT": shared["kvT"],
        "mk": shared["mk"],
        "ident": shared["ident"],
        "indh": shared["indh"],
        "wq": packw(Wq, 1.0 / np.sqrt(DH)),
        "wk": packw(Wk),
        "wv": packw(Wv),
        "wo": np.ascontiguousarray(Wo[:, rows].T, dtype=np.float32),
        "pb": pbp,
    }
    return m


def _prep_shared(query, key_value, mask):
    qTp = np.ascontiguousarray(
        query.reshape(B, L, DC, 128).transpose(0, 2, 3, 1), dtype=np.float32
    )
    kvTp = np.ascontiguousarray(
        key_value.reshape(B, L, DC, 128).transpose(0, 2, 3, 1), dtype=np.float32
    )
    mku = (~np.asarray(mask, dtype=bool)).astype(np.uint8)  # inverted: 1 = masked out
    # [b, q, k] -> [qt, kp, b, kt, qf]
    mkp = np.ascontiguousarray(
        mku.reshape(B, QN, QTS, KN, 128).transpose(1, 4, 0, 3, 2)
    )
    return {
        "qT": qTp,
        "kvT": kvTp,
        "mk": mkp,
        "ident": np.eye(128, dtype=np.float32),
        "indh": np.ascontiguousarray(
            np.concatenate(
                [
                    np.where(np.arange(128) < 64, 1.0, 0.0),
                    np.where(np.arange(128) >= 64, 1.0, 0.0),
                ]
            ).astype(np.float32)[None, :]
        ),
    }


def kernel(query, key_value, mask, position_bias, Wq, Wk, Wv, Wo, _trace=False):
    query = np.asarray(query, dtype=np.float32)
    key_value = np.asarray(key_value, dtype=np.float32)
    mask = np.asarray(mask)
    position_bias = np.asarray(position_bias, dtype=np.float32)
    Wq = np.asarray(Wq, dtype=np.float32)
    Wk = np.asarray(Wk, dtype=np.float32)
    Wv = np.asarray(Wv, dtype=np.float32)
    Wo = np.asarray(Wo, dtype=np.float32)

    nc = _build()
    shared = _prep_shared(query, key_value, mask)
    in_maps = [
        _prep_core(c, query, key_value, mask, position_bias, Wq, Wk, Wv, Wo, shared)
        for c in range(N_CORES)
    ]
    res = run_bass_kernel_spmd(
        nc, in_maps, list(range(N_CORES)), trace=_trace
    )
    _CACHE["last_result"] = res
    acc = res.results[0]["out"].astype(np.float64)
    for c in range(1, N_CORES):
        acc += res.results[c]["out"]
    return acc.astype(np.float32)


# revision 21
# speedup vs baseline: 1.0024x; 1.0024x over previous
"""CPM3 attention kernel for 8 trn2 NeuronCores.

Sharding: tensor-parallel over heads (2 heads/core x both batches).
Device computes per-core partial outputs (Wo row-sharded); host sums.
"""

import sys

sys.path.insert(0, "/opt/trn_rl_repo")

import numpy as np
import ml_dtypes

import concourse.bass as bass
import concourse.bacc as bacc
import concourse.tile as tile
import concourse.mybir as mybir
from concourse.bass_utils import run_bass_kernel_spmd

B, L, D, H, DH = 2, 2048, 1024, 16, 64
N_CORES = 8
HPC = H // N_CORES  # heads per core = 2
D2H = HPC * DH  # 128
QTS = 512  # q tile size
QN = L // QTS  # 4
KP = 128  # k partition tile
KN = L // KP  # 16
KTG = 4  # k tiles per DMA group
KGN = KN // KTG  # 4
DC = D // 128  # 8 contraction chunks
HVW = 2 * (DH + 1)  # 130: hv_aug columns per k-tile (2 heads x (64+ones))

F32 = mybir.dt.float32
F32R = mybir.dt.float32r
U8 = mybir.dt.uint8
BF16 = mybir.dt.bfloat16
MM_NEG = 1.0e9  # additive mask magnitude

_CACHE: dict = {}


def _build():
    if "nc" in _CACHE:
        return _CACHE["nc"]
    nc = bacc.Bacc("TRN2", target_bir_lowering=False, debug=False, num_devices=N_CORES)

    qT = nc.dram_tensor("qT", [B, DC, 128, L], F32R, kind="ExternalInput").ap()
    kvT = nc.dram_tensor("kvT", [B, DC, 128, L], F32R, kind="ExternalInput").ap()
    wq = nc.dram_tensor("wq", [128, DC, 128], F32R, kind="ExternalInput").ap()
    wk = nc.dram_tensor("wk", [128, DC, 128], F32R, kind="ExternalInput").ap()
    wv = nc.dram_tensor("wv", [128, DC, 128], F32R, kind="ExternalInput").ap()
    wo = nc.dram_tensor("wo", [128, D], F32R, kind="ExternalInput").ap()
    pb = nc.dram_tensor(
        "pb", [QN, KGN, 128, KTG, HPC, QTS], F32R, kind="ExternalInput"
    ).ap()
    mk = nc.dram_tensor("mk", [QN, 128, B, KN, QTS], U8, kind="ExternalInput").ap()
    ident = nc.dram_tensor("ident", [128, 128], F32R, kind="ExternalInput").ap()
    indh = nc.dram_tensor("indh", [1, 256], F32R, kind="ExternalInput").ap()
    out = nc.dram_tensor("out", [B, L, D], F32, kind="ExternalOutput").ap()

    with tile.TileContext(nc) as tc:
        with (
            tc.tile_pool(name="const", bufs=1) as constp,
            tc.tile_pool(name="hq", bufs=2) as hqp,
            tc.tile_pool(name="hk", bufs=2) as hkp,
            tc.tile_pool(name="hv", bufs=2) as hvp,
            tc.tile_pool(name="stage", bufs=2) as stagep,
            tc.tile_pool(name="pbp", bufs=2) as pbp,
            tc.tile_pool(name="mask", bufs=2) as mkp,
            tc.tile_pool(name="ma", bufs=8) as map_,
            tc.tile_pool(name="pt", bufs=10) as ptp,
            tc.tile_pool(name="ctxn", bufs=2) as ctxnp,
            tc.tile_pool(name="rc", bufs=2) as rcp,
            tc.tile_pool(name="outb", bufs=3) as outp,
            tc.tile_pool(name="psum", bufs=8, space=bass.MemorySpace.PSUM) as psp,
        ):
            # ---- constants ----
            ident_t = constp.tile([128, 128], F32R, tag="ident")
            nc.sync.dma_start(ident_t[:], ident[:])
            indh_t = constp.tile([1, 256], F32R, tag="indh")
            nc.sync.dma_start(indh_t[:], indh[:])
            wq_t = constp.tile([128, DC, 128], F32R, tag="wq")
            nc.sync.dma_start(wq_t[:], wq[:])
            wk_t = constp.tile([128, DC, 128], F32R, tag="wk")
            nc.sync.dma_start(wk_t[:], wk[:])
            wv_t = constp.tile([128, DC, 128], F32R, tag="wv")
            nc.sync.dma_start(wv_t[:], wv[:])
            wo_t = constp.tile([128, D], F32R, tag="wo")
            nc.sync.dma_start(wo_t[:], wo[:])

            # ---- prologue: projections ----
            hq_sb, hk_sb, hv_sb = {}, {}, {}
            for b in range(B):
                hq_sb[b] = hqp.tile([128, L], F32R, tag="hq", name=f"hq_sb{b}")
                hq_ps = [psp.tile([128, QTS], F32, tag="bank", name=f"hq_ps{b}_{i}") for i in range(QN)]
                for dc in range(DC):
                    qc = stagep.tile([128, L], F32R, tag="stage")
                    nc.sync.dma_start(qc[:], qT[b, dc])
                    for qt in range(QN):
                        nc.tensor.matmul(
                            hq_ps[qt][:],
                            wq_t[:, dc, :],
                            qc[:, qt * QTS : (qt + 1) * QTS],
                            start=(dc == 0),
                            stop=(dc == DC - 1),
                        )
                for qt in range(QN):
                    nc.vector.tensor_copy(
                        hq_sb[b][:, qt * QTS : (qt + 1) * QTS],
                        hq_ps[qt][:],
                    )

                hk_sb[b] = hkp.tile([128, L], F32R, tag="hk", name=f"hk_sb{b}")
                hvT = stagep.tile([128, L], F32R, tag="stage")
                hk_ps = [psp.tile([128, QTS], F32, tag="bank", name=f"hk_ps{b}_{i}") for i in range(QN)]
                hv_ps = [psp.tile([128, QTS], F32, tag="bank", name=f"hv_ps{b}_{i}") for i in range(QN)]
                for dc in range(DC):
                    kc = stagep.tile([128, L], F32R, tag="stage")
                    nc.sync.dma_start(kc[:], kvT[b, dc])
                    for qt in range(QN):
                        nc.tensor.matmul(
                            hk_ps[qt][:],
                            wk_t[:, dc, :],
                            kc[:, qt * QTS : (qt + 1) * QTS],
                            start=(dc == 0),
                            stop=(dc == DC - 1),
                        )
                        nc.tensor.matmul(
                            hv_ps[qt][:],
                            wv_t[:, dc, :],
                            kc[:, qt * QTS : (qt + 1) * QTS],
                            start=(dc == 0),
                            stop=(dc == DC - 1),
                        )
                for qt in range(QN):
                    nc.vector.tensor_copy(
                        hk_sb[b][:, qt * QTS : (qt + 1) * QTS],
                        hk_ps[qt][:],
                    )
                    nc.vector.tensor_copy(
                        hvT[:, qt * QTS : (qt + 1) * QTS], hv_ps[qt][:]
                    )

                # hv_aug: transpose hvT per k-tile; ones cols prefilled
                hv_sb[b] = hvp.tile([128, KN * HVW], F32R, tag="hv", name=f"hv_sb{b}")
                nc.gpsimd.memset(hv_sb[b][:].bitcast(mybir.dt.uint32), 0x3F800000)
                for kt in range(KN):
                    tp = psp.tile([128, 128], F32R, tag="bank")
                    nc.tensor.transpose(
                        tp[:], hvT[:, kt * KP : (kt + 1) * KP], ident_t[:]
                    )
                    o = kt * HVW
                    nc.vector.tensor_copy(hv_sb[b][:, o : o + DH], tp[:, 0:DH])
                    nc.vector.tensor_copy(
                        hv_sb[b][:, o + DH + 1 : o + 2 * DH + 1], tp[:, DH:128]
                    )

            # ---- main loop ----
            for qt in range(QN):
                mk_t = {}
                for b in range(B):
                    mk_t[b] = mkp.tile(
                        [128, KN, QTS], U8, tag="mask", name=f"mk_t{b}_{qt}"
                    )
                    nc.scalar.dma_start(mk_t[b][:], mk[qt, :, b])
                ctx_ps = {}
                for b in range(B):
                    for h in range(HPC):
                        ctx_ps[(b, h)] = psp.tile(
                            [DH + 1, QTS], F32, tag="bank", name=f"ctx_ps{b}_{h}_{qt}"
                        )
                pending_pv = []
                for kg in range(KGN):
                    pb_t = pbp.tile([128, KTG, HPC, QTS], F32R, tag="pb")
                    nc.scalar.dma_start(pb_t[:], pb[qt, kg])
                    for ki in range(KTG):
                        kt = kg * KTG + ki
                        new_pv = []
                        for b in range(B):
                            for h in range(HPC):
                                cmb = map_.tile(
                                    [128, QTS], F32R, tag="ma", name=f"cmb{b}_{h}_{kt}"
                                )
                                nc.vector.scalar_tensor_tensor(
                                    cmb[:],
                                    mk_t[b][:, kt, :],
                                    -MM_NEG,
                                    pb_t[:, ki, h, :],
                                    mybir.AluOpType.mult,
                                    mybir.AluOpType.add,
                                )
                                sc = psp.tile(
                                    [128, QTS], F32, tag="bank", name=f"sc{b}_{h}_{kt}"
                                )
                                nc.tensor.matmul(
                                    sc[:], ident_t[:], cmb[:], start=True, stop=False
                                )
                                nc.tensor.matmul(
                                    sc[:],
                                    hk_sb[b][h * DH : (h + 1) * DH, kt * KP : (kt + 1) * KP],
                                    hq_sb[b][h * DH : (h + 1) * DH, qt * QTS : (qt + 1) * QTS],
                                    start=False,
                                    stop=True,
                                )
                                p_t = ptp.tile(
                                    [128, QTS], F32R, tag="pt", name=f"p_t{b}_{h}_{kt}"
                                )
                                nc.scalar.activation(
                                    p_t[:], sc[:], mybir.ActivationFunctionType.Exp
                                )
                                new_pv.append((b, h, kt, p_t))
                        # software pipeline: PV of previous k-tile runs now, when
                        # its exp has certainly finished (PE is in-order)
                        for b, h, pkt, p_t in pending_pv:
                            o = pkt * HVW + h * (DH + 1)
                            nc.tensor.matmul(
                                ctx_ps[(b, h)][:],
                                hv_sb[b][:, o : o + DH + 1],
                                p_t[:],
                                start=(pkt == 0),
                                stop=(pkt == KN - 1),
                            )
                        pending_pv = new_pv
                for b, h, pkt, p_t in pending_pv:
                    o = pkt * HVW + h * (DH + 1)
                    nc.tensor.matmul(
                        ctx_ps[(b, h)][:],
                        hv_sb[b][:, o : o + DH + 1],
                        p_t[:],
                        start=(pkt == 0),
                        stop=(pkt == KN - 1),
                    )
                # normalize + output projection
                for b in range(B):
                    ctxn = ctxnp.tile([128, QTS], F32R, tag="ctxn")
                    bcw = psp.tile([128, QTS], F32, tag="bank", name=f"bcw{b}")
                    bc = bcw[:]
                    for h in range(HPC):
                        dsb = rcp.tile([1, QTS], F32, tag="dsb", name=f"dsb{b}_{h}")
                        nc.vector.tensor_copy(dsb[:], ctx_ps[(b, h)][DH : DH + 1, :])
                        rcf = rcp.tile([1, QTS], F32, tag="rcf", name=f"rcf{b}_{h}")
                        nc.vector.reciprocal_approx_fast(rcf[:], dsb[:])
                        rcr = rcp.tile([1, QTS], F32R, tag="rcr", name=f"rcr{b}_{h}")
                        nc.vector.tensor_copy(rcr[:], rcf[:])
                        nc.tensor.matmul(
                            bc,
                            indh_t[:, h * 128 : (h + 1) * 128],
                            rcr[:],
                            start=(h == 0),
                            stop=(h == HPC - 1),
                        )
                    bc_sb = rcp.tile([128, QTS], F32, tag="bcsb", name=f"bc_sb{b}")
                    nc.vector.tensor_copy(bc_sb[:], bc)
                    for h in range(HPC):
                        nc.vector.tensor_tensor(
                            ctxn[h * DH : (h + 1) * DH, :],
                            ctx_ps[(b, h)][0:DH, :],
                            bc_sb[h * DH : (h + 1) * DH, :],
                            mybir.AluOpType.mult,
                        )
                    for qs in range(QN):
                        ob = outp.tile([128, D], F32, tag="outb")
                        for oh in range(2):
                            op_ps = psp.tile(
                                [128, QTS], F32, tag="bank", name=f"op{b}_{qs}_{oh}"
                            )
                            nc.tensor.matmul(
                                op_ps[:],
                                ctxn[:, qs * 128 : (qs + 1) * 128],
                                wo_t[:, oh * QTS : (oh + 1) * QTS],
                                start=True,
                                stop=True,
                            )
                            nc.scalar.copy(
                                ob[:, oh * QTS : (oh + 1) * QTS], op_ps[:]
                            )
                        r0 = qt * QTS + qs * 128
                        nc.sync.dma_start(out[b, r0 : r0 + 128, :], ob[:])

    nc.compile()
    _CACHE["nc"] = nc
    return nc


def _prep_core(core, query, key_value, mask, position_bias, Wq, Wk, Wv, Wo, shared):
    """Per-core input map. `shared` holds core-independent packed arrays."""
    h0 = core * HPC
    rows = slice(h0 * DH, (h0 + HPC) * DH)
    # weight chunks packed [dp, dc, d2h] from W[rows].T [D, 128]
    def packw(w, scale=1.0):
        wt = np.ascontiguousarray(
            (w[rows].T * scale).reshape(DC, 128, 128).transpose(1, 0, 2),
            dtype=np.float32,
        )
        return wt

    pbc = position_bias[h0 : h0 + HPC]  # [2, q, k]
    # -> [qt, kg, kp, ki, h, qf]
    pbp = np.ascontiguousarray(
        pbc.reshape(HPC, QN, QTS, KGN, KTG, 128).transpose(1, 3, 5, 4, 0, 2),
        dtype=np.float32,
    )
    m = {
        "qT": shared["qT"],
        "kvT": shared["kvT"],
        "mk": shared["mk"],
        "ident": shared["ident"],
        "indh": shared["indh"],
        "wq": packw(Wq, 1.0 / np.sqrt(DH)),
        "wk": packw(Wk),
        "wv": packw(Wv),
        "wo": np.ascontiguousarray(Wo[:, rows].T, dtype=np.float32),
        "pb": pbp,
    }
    return m


def _prep_shared(query, key_value, mask):
    qTp = np.ascontiguousarray(
        query.reshape(B, L, DC, 128).transpose(0, 2, 3, 1), dtype=np.float32
    )
    kvTp = np.ascontiguousarray(
        key_value.reshape(B, L, DC, 128).transpose(0, 2, 3, 1), dtype=np.float32
    )
    mku = (~np.asarray(mask, dtype=bool)).astype(np.uint8)  # inverted: 1 = masked out
    # [b, q, k] -> [qt, kp, b, kt, qf]
    mkp = np.ascontiguousarray(
        mku.reshape(B, QN, QTS, KN, 128).transpose(1, 4, 0, 3, 2)
    )
    return {
        "qT": qTp,
        "kvT": kvTp,
        "mk": mkp,
        "ident": np.eye(128, dtype=np.float32),
        "indh": np.ascontiguousarray(
            np.concatenate(
                [
                    np.where(np.arange(128) < 64, 1.0, 0.0),
                    np.where(np.arange(128) >= 64, 1.0, 0.0),
                ]
            ).astype(np.float32)[None, :]
        ),
    }


def kernel(query, key_value, mask, position_bias, Wq, Wk, Wv, Wo, _trace=False):
    query = np.asarray(query, dtype=np.float32)
    key_value = np.asarray(key_value, dtype=np.float32)
    mask = np.asarray(mask)
    position_bias = np.asarray(position_bias, dtype=np.float32)
    Wq = np.asarray(Wq, dtype=np.float32)
    Wk = np.asarray(Wk, dtype=np.float32)
    Wv = np.asarray(Wv, dtype=np.float32)
    Wo = np.asarray(Wo, dtype=np.float32)

    nc = _build()
    shared = _prep_shared(query, key_value, mask)
    in_maps = [
        _prep_core(c, query, key_value, mask, position_bias, Wq, Wk, Wv, Wo, shared)
        for c in range(N_CORES)
    ]
    res = run_bass_kernel_spmd(
        nc, in_maps, list(range(N_CORES)), trace=_trace
    )
    _CACHE["last_result"] = res
    acc = res.results[0]["out"].astype(np.float64)
    for c in range(1, N_CORES):
        acc += res.results[c]["out"]
    return acc.astype(np.float32)


# revision 24
# speedup vs baseline: 1.6227x; 1.6188x over previous
"""CPM3 attention kernel for 8 trn2 NeuronCores.

Sharding: tensor-parallel over heads (2 heads/core x both batches).
Device computes per-core partial outputs (Wo row-sharded); host sums.

Data layout tricks:
- host pre-transposes q/kv/position_bias/mask so the device never transposes
  big tensors; scores are computed transposed [k, q] so the softmax needs no
  partition-dim reductions (a ones-column in V yields the denominators).
- fp16 operands for all matmuls: 2-byte weights use the PE background
  weight-load path (4-byte fp32r serializes LDWEIGHTS per matmul) and halve
  HBM traffic. PSUM accumulation stays fp32.
- position_bias + additive mask are fused into one DVE scalar_tensor_tensor
  and enter PSUM via a single identity matmul per score tile.
"""

import sys

sys.path.insert(0, "/opt/trn_rl_repo")

import numpy as np
import ml_dtypes

import concourse.bass as bass
import concourse.bacc as bacc
import concourse.tile as tile
import concourse.mybir as mybir
from concourse.bass_utils import run_bass_kernel_spmd

B, L, D, H, DH = 2, 2048, 1024, 16, 64
N_CORES = 8
HPC = H // N_CORES  # heads per core = 2
QTS = 512  # q tile size
QN = L // QTS  # 4
KP = 128  # k partition tile
KN = L // KP  # 16
KTG = 4  # k tiles per DMA group
KGN = KN // KTG  # 4
DC = D // 128  # 8 contraction chunks
HVW = 2 * (DH + 1)  # 130: hv_aug columns per k-tile (2 heads x (64+ones))

F32 = mybir.dt.float32
F32R = mybir.dt.float32r
F16 = mybir.dt.float16
U8 = mybir.dt.uint8
MM_NEG = 60000.0  # additive mask magnitude (fits fp16)

_CACHE: dict = {}


def _build():
    if "nc" in _CACHE:
        return _CACHE["nc"]
    nc = bacc.Bacc("TRN2", target_bir_lowering=False, debug=False, num_devices=N_CORES)

    qT = nc.dram_tensor("qT", [B, DC, 128, L], F16, kind="ExternalInput").ap()
    kvT = nc.dram_tensor("kvT", [B, DC, 128, L], F16, kind="ExternalInput").ap()
    wq = nc.dram_tensor("wq", [128, DC, 128], F16, kind="ExternalInput").ap()
    wk = nc.dram_tensor("wk", [128, DC, 128], F16, kind="ExternalInput").ap()
    wv = nc.dram_tensor("wv", [128, DC, 128], F16, kind="ExternalInput").ap()
    wo = nc.dram_tensor("wo", [128, D], F16, kind="ExternalInput").ap()
    pb = nc.dram_tensor(
        "pb", [QN, KGN, 128, KTG, HPC, QTS], F16, kind="ExternalInput"
    ).ap()
    mk = nc.dram_tensor("mk", [QN, 128, B, KN, QTS], U8, kind="ExternalInput").ap()
    ident = nc.dram_tensor("ident", [128, 128], F16, kind="ExternalInput").ap()
    identr = nc.dram_tensor("identr", [128, 128], F32R, kind="ExternalInput").ap()
    indh = nc.dram_tensor("indh", [1, 256], F16, kind="ExternalInput").ap()
    out = nc.dram_tensor("out", [B, L, D], F32, kind="ExternalOutput").ap()

    with tile.TileContext(nc) as tc:
        with (
            tc.tile_pool(name="const", bufs=1) as constp,
            tc.tile_pool(name="hq", bufs=2) as hqp,
            tc.tile_pool(name="hk", bufs=2) as hkp,
            tc.tile_pool(name="hv", bufs=2) as hvp,
            tc.tile_pool(name="stage", bufs=3) as stagep,
            tc.tile_pool(name="pbp", bufs=2) as pbp,
            tc.tile_pool(name="mask", bufs=2) as mkp,
            tc.tile_pool(name="ma", bufs=8) as map_,
            tc.tile_pool(name="pt", bufs=10) as ptp,
            tc.tile_pool(name="ctxn", bufs=2) as ctxnp,
            tc.tile_pool(name="rc", bufs=2) as rcp,
            tc.tile_pool(name="outb", bufs=3) as outp,
            tc.tile_pool(name="psum", bufs=8, space=bass.MemorySpace.PSUM) as psp,
        ):
            # ---- constants ----
            ident_t = constp.tile([128, 128], F16, tag="ident")
            nc.sync.dma_start(ident_t[:], ident[:])
            identr_t = constp.tile([128, 128], F32R, tag="identr")
            nc.sync.dma_start(identr_t[:], identr[:])
            indh_t = constp.tile([1, 256], F16, tag="indh")
            nc.sync.dma_start(indh_t[:], indh[:])
            wq_t = constp.tile([128, DC, 128], F16, tag="wq")
            nc.sync.dma_start(wq_t[:], wq[:])
            wk_t = constp.tile([128, DC, 128], F16, tag="wk")
            nc.sync.dma_start(wk_t[:], wk[:])
            wv_t = constp.tile([128, DC, 128], F16, tag="wv")
            nc.sync.dma_start(wv_t[:], wv[:])
            wo_t = constp.tile([128, D], F16, tag="wo")
            nc.sync.dma_start(wo_t[:], wo[:])

            # ---- prologue: projections ----
            hq_sb, hk_sb, hv_sb = {}, {}, {}
            for b in range(B):
                hq_sb[b] = hqp.tile([128, L], F16, tag="hq", name=f"hq_sb{b}")
                hq_ps = [
                    psp.tile([128, QTS], F32, tag="bank", name=f"hq_ps{b}_{i}")
                    for i in range(QN)
                ]
                for dc in range(DC):
                    qc = stagep.tile([128, L], F16, tag="stage")
                    nc.sync.dma_start(qc[:], qT[b, dc])
                    for qt in range(QN):
                        nc.tensor.matmul(
                            hq_ps[qt][:],
                            wq_t[:, dc, :],
                            qc[:, qt * QTS : (qt + 1) * QTS],
                            start=(dc == 0),
                            stop=(dc == DC - 1),
                        )
                for qt in range(QN):
                    nc.vector.tensor_copy(
                        hq_sb[b][:, qt * QTS : (qt + 1) * QTS], hq_ps[qt][:]
                    )

                hk_sb[b] = hkp.tile([128, L], F16, tag="hk", name=f"hk_sb{b}")
                hvT = stagep.tile([128, L], F32R, tag="stage")
                hk_ps = [
                    psp.tile([128, QTS], F32, tag="bank", name=f"hk_ps{b}_{i}")
                    for i in range(QN)
                ]
                hv_ps = [
                    psp.tile([128, QTS], F32, tag="bank", name=f"hv_ps{b}_{i}")
                    for i in range(QN)
                ]
                for dc in range(DC):
                    kc = stagep.tile([128, L], F16, tag="stage")
                    nc.sync.dma_start(kc[:], kvT[b, dc])
                    for qt in range(QN):
                        nc.tensor.matmul(
                            hk_ps[qt][:],
                            wk_t[:, dc, :],
                            kc[:, qt * QTS : (qt + 1) * QTS],
                            start=(dc == 0),
                            stop=(dc == DC - 1),
                        )
                        nc.tensor.matmul(
                            hv_ps[qt][:],
                            wv_t[:, dc, :],
                            kc[:, qt * QTS : (qt + 1) * QTS],
                            start=(dc == 0),
                            stop=(dc == DC - 1),
                        )
                for qt in range(QN):
                    nc.vector.tensor_copy(
                        hk_sb[b][:, qt * QTS : (qt + 1) * QTS], hk_ps[qt][:]
                    )
                    nc.vector.tensor_copy(
                        hvT[:, qt * QTS : (qt + 1) * QTS], hv_ps[qt][:]
                    )

                # hv_aug: transpose hvT per k-tile; ones cols prefilled
                hv_sb[b] = hvp.tile([128, KN * HVW], F16, tag="hv", name=f"hv_sb{b}")
                nc.gpsimd.memset(hv_sb[b][:].bitcast(mybir.dt.uint16), 0x3C00)
                for kt in range(KN):
                    tp = psp.tile([128, 128], F32R, tag="bank")
                    nc.tensor.transpose(
                        tp[:], hvT[:, kt * KP : (kt + 1) * KP], identr_t[:]
                    )
                    o = kt * HVW
                    nc.vector.tensor_copy(hv_sb[b][:, o : o + DH], tp[:, 0:DH])
                    nc.vector.tensor_copy(
                        hv_sb[b][:, o + DH + 1 : o + 2 * DH + 1], tp[:, DH:128]
                    )

            # ---- main loop ----
            for qt in range(QN):
                mk_t = {}
                for b in range(B):
                    mk_t[b] = mkp.tile(
                        [128, KN, QTS], U8, tag="mask", name=f"mk_t{b}_{qt}"
                    )
                    nc.scalar.dma_start(mk_t[b][:], mk[qt, :, b])
                ctx_ps = {}
                for b in range(B):
                    for h in range(HPC):
                        ctx_ps[(b, h)] = psp.tile(
                            [DH + 1, QTS], F32, tag="bank", name=f"ctx_ps{b}_{h}_{qt}"
                        )
                pending_pv = []
                for kg in range(KGN):
                    pb_t = pbp.tile([128, KTG, HPC, QTS], F16, tag="pb")
                    nc.scalar.dma_start(pb_t[:], pb[qt, kg])
                    for ki in range(KTG):
                        kt = kg * KTG + ki
                        # bias+mask fused on DVE
                        cmb, sc = {}, {}
                        for b in range(B):
                            for h in range(HPC):
                                cmb[(b, h)] = map_.tile(
                                    [128, QTS], F16, tag="ma", name=f"cmb{b}_{h}_{kt}"
                                )
                                nc.vector.scalar_tensor_tensor(
                                    cmb[(b, h)][:],
                                    mk_t[b][:, kt, :],
                                    -MM_NEG,
                                    pb_t[:, ki, h, :],
                                    mybir.AluOpType.mult,
                                    mybir.AluOpType.add,
                                )
                        # identity matmuls: same stationary, batched
                        for b in range(B):
                            for h in range(HPC):
                                sc[(b, h)] = psp.tile(
                                    [128, QTS], F32, tag="bank", name=f"sc{b}_{h}_{kt}"
                                )
                                nc.tensor.matmul(
                                    sc[(b, h)][:],
                                    ident_t[:],
                                    cmb[(b, h)][:],
                                    start=True,
                                    stop=False,
                                )
                        # QK: h0 rows 0-63 / h1 rows 64-127 run concurrently
                        for b in range(B):
                            for h in range(HPC):
                                nc.tensor.matmul(
                                    sc[(b, h)][:],
                                    hk_sb[b][
                                        h * DH : (h + 1) * DH, kt * KP : (kt + 1) * KP
                                    ],
                                    hq_sb[b][
                                        h * DH : (h + 1) * DH,
                                        qt * QTS : (qt + 1) * QTS,
                                    ],
                                    start=False,
                                    stop=True,
                                )
                        new_pv = []
                        for b in range(B):
                            for h in range(HPC):
                                p_t = ptp.tile(
                                    [128, QTS], F16, tag="pt", name=f"p_t{b}_{h}_{kt}"
                                )
                                nc.scalar.activation(
                                    p_t[:],
                                    sc[(b, h)][:],
                                    mybir.ActivationFunctionType.Exp,
                                )
                                new_pv.append((b, h, kt, p_t))
                        # software pipeline: PV of previous k-tile (its exp is
                        # certainly done; PE is in-order)
                        for b, h, pkt, p_t in pending_pv:
                            o = pkt * HVW + h * (DH + 1)
                            nc.tensor.matmul(
                                ctx_ps[(b, h)][:],
                                hv_sb[b][:, o : o + DH + 1],
                                p_t[:],
                                start=(pkt == 0),
                                stop=(pkt == KN - 1),
                            )
                        pending_pv = new_pv
                for b, h, pkt, p_t in pending_pv:
                    o = pkt * HVW + h * (DH + 1)
                    nc.tensor.matmul(
                        ctx_ps[(b, h)][:],
                        hv_sb[b][:, o : o + DH + 1],
                        p_t[:],
                        start=(pkt == 0),
                        stop=(pkt == KN - 1),
                    )
                # normalize + output projection
                for b in range(B):
                    ctxn = ctxnp.tile([128, QTS], F16, tag="ctxn")
                    bcw = psp.tile([128, QTS], F32, tag="bank", name=f"bcw{b}")
                    bc = bcw[:]
                    for h in range(HPC):
                        dsb = rcp.tile([1, QTS], F32, tag="dsb", name=f"dsb{b}_{h}")
                        nc.vector.tensor_copy(dsb[:], ctx_ps[(b, h)][DH : DH + 1, :])
                        rcf = rcp.tile([1, QTS], F32, tag="rcf", name=f"rcf{b}_{h}")
                        nc.vector.reciprocal_approx_fast(rcf[:], dsb[:])
                        rcr = rcp.tile([1, QTS], F16, tag="rcr", name=f"rcr{b}_{h}")
                        nc.vector.tensor_copy(rcr[:], rcf[:])
                        nc.tensor.matmul(
                            bc,
                            indh_t[:, h * 128 : (h + 1) * 128],
                            rcr[:],
                            start=(h == 0),
                            stop=(h == HPC - 1),
                        )
                    bc_sb = rcp.tile([128, QTS], F32, tag="bcsb", name=f"bc_sb{b}")
                    nc.vector.tensor_copy(bc_sb[:], bc)
                    for h in range(HPC):
                        nc.vector.tensor_tensor(
                            ctxn[h * DH : (h + 1) * DH, :],
                            ctx_ps[(b, h)][0:DH, :],
                            bc_sb[h * DH : (h + 1) * DH, :],
                            mybir.AluOpType.mult,
                        )
                    for qs in range(QN):
                        ob = outp.tile([128, D], F32, tag="outb")
                        for oh in range(2):
                            op_ps = psp.tile(
                                [128, QTS], F32, tag="bank", name=f"op{b}_{qs}_{oh}"
                            )
                            nc.tensor.matmul(
                                op_ps[:],
                                ctxn[:, qs * 128 : (qs + 1) * 128],
                                wo_t[:, oh * QTS : (oh + 1) * QTS],
                                start=True,
                                stop=True,
                            )
                            nc.scalar.copy(ob[:, oh * QTS : (oh + 1) * QTS], op_ps[:])
                        r0 = qt * QTS + qs * 128
                        nc.sync.dma_start(out[b, r0 : r0 + 128, :], ob[:])

    nc.compile()
    _CACHE["nc"] = nc
    return nc


def _prep_core(core, query, key_value, mask, position_bias, Wq, Wk, Wv, Wo, shared):
    """Per-core input map. `shared` holds core-independent packed arrays."""
    h0 = core * HPC
    rows = slice(h0 * DH, (h0 + HPC) * DH)

    def packw(w, scale=1.0):
        return np.ascontiguousarray(
            (w[rows].T * scale).reshape(DC, 128, 128).transpose(1, 0, 2)
        ).astype(np.float16)

    pbc = position_bias[h0 : h0 + HPC]  # [2, q, k]
    # -> [qt, kg, kp, ki, h, qf]
    pbp = np.ascontiguousarray(
        pbc.reshape(HPC, QN, QTS, KGN, KTG, 128).transpose(1, 3, 5, 4, 0, 2)
    ).astype(np.float16)
    return {
        "qT": shared["qT"],
        "kvT": shared["kvT"],
        "mk": shared["mk"],
        "ident": shared["ident"],
        "identr": shared["identr"],
        "indh": shared["indh"],
        "wq": packw(Wq, 1.0 / np.sqrt(DH)),
        "wk": packw(Wk),
        "wv": packw(Wv),
        "wo": np.ascontiguousarray(Wo[:, rows].T).astype(np.float16),
        "pb": pbp,
    }


def _prep_shared(query, key_value, mask):
    qTp = np.ascontiguousarray(
        query.reshape(B, L, DC, 128).transpose(0, 2, 3, 1)
    ).astype(np.float16)
    kvTp = np.ascontiguousarray(
        key_value.reshape(B, L, DC, 128).transpose(0, 2, 3, 1)
    ).astype(np.float16)
    mku = (~np.asarray(mask, dtype=bool)).astype(np.uint8)  # 1 = masked out
    # [b, q, k] -> [qt, kp, b, kt, qf]
    mkp = np.ascontiguousarray(
        mku.reshape(B, QN, QTS, KN, 128).transpose(1, 4, 0, 3, 2)
    )
    indh = np.concatenate(
        [
            np.where(np.arange(128) < 64, 1.0, 0.0),
            np.where(np.arange(128) >= 64, 1.0, 0.0),
        ]
    ).astype(np.float16)[None, :]
    return {
        "qT": qTp,
        "kvT": kvTp,
        "mk": mkp,
        "ident": np.eye(128, dtype=np.float16),
        "identr": np.eye(128, dtype=np.float32),
        "indh": np.ascontiguousarray(indh),
    }


def kernel(query, key_value, mask, position_bias, Wq, Wk, Wv, Wo, _trace=False):
    query = np.asarray(query, dtype=np.float32)
    key_value = np.asarray(key_value, dtype=np.float32)
    mask = np.asarray(mask)
    position_bias = np.asarray(position_bias, dtype=np.float32)
    Wq = np.asarray(Wq, dtype=np.float32)
    Wk = np.asarray(Wk, dtype=np.float32)
    Wv = np.asarray(Wv, dtype=np.float32)
    Wo = np.asarray(Wo, dtype=np.float32)

    nc = _build()
    shared = _prep_shared(query, key_value, mask)
    in_maps = [
        _prep_core(c, query, key_value, mask, position_bias, Wq, Wk, Wv, Wo, shared)
        for c in range(N_CORES)
    ]
    res = run_bass_kernel_spmd(nc, in_maps, list(range(N_CORES)), trace=_trace)
    _CACHE["last_result"] = res
    acc = res.results[0]["out"].astype(np.float64)
    for c in range(1, N_CORES):
        acc += res.results[c]["out"]
    return acc.astype(np.float32)


# revision 26
# speedup vs baseline: 1.6373x; 1.0090x over previous
"""CPM3 attention kernel for 8 trn2 NeuronCores.

Sharding: tensor-parallel over heads (2 heads/core x both batches).
Device computes per-core partial outputs (Wo row-sharded); host sums.

Data layout tricks:
- host pre-transposes q/kv/position_bias/mask so the device never transposes
  big tensors; scores are computed transposed [k, q] so the softmax needs no
  partition-dim reductions (a ones-column in V yields the denominators).
- fp16 operands for all matmuls: 2-byte weights use the PE background
  weight-load path (4-byte fp32r serializes LDWEIGHTS per matmul) and halve
  HBM traffic. PSUM accumulation stays fp32.
- position_bias + additive mask are fused into one DVE scalar_tensor_tensor
  and enter PSUM via a single identity matmul per score tile.
"""

import sys

sys.path.insert(0, "/opt/trn_rl_repo")

import numpy as np
import ml_dtypes

import concourse.bass as bass
import concourse.bacc as bacc
import concourse.tile as tile
import concourse.mybir as mybir
from concourse.bass_utils import run_bass_kernel_spmd

B, L, D, H, DH = 2, 2048, 1024, 16, 64
N_CORES = 8
HPC = H // N_CORES  # heads per core = 2
QTS = 512  # q tile size
QN = L // QTS  # 4
KP = 128  # k partition tile
KN = L // KP  # 16
KTG = 4  # k tiles per DMA group
KGN = KN // KTG  # 4
DC = D // 128  # 8 contraction chunks
HVW = 2 * (DH + 1)  # 130: hv_aug columns per k-tile (2 heads x (64+ones))

F32 = mybir.dt.float32
F32R = mybir.dt.float32r
F16 = mybir.dt.float16
U8 = mybir.dt.uint8
MM_NEG = 60000.0  # additive mask magnitude (fits fp16)

_CACHE: dict = {}


def _build():
    if "nc" in _CACHE:
        return _CACHE["nc"]
    nc = bacc.Bacc("TRN2", target_bir_lowering=False, debug=False, num_devices=N_CORES)

    qT = nc.dram_tensor("qT", [B, DC, 128, L], F16, kind="ExternalInput").ap()
    kvT = nc.dram_tensor("kvT", [B, DC, 128, L], F16, kind="ExternalInput").ap()
    wq = nc.dram_tensor("wq", [128, DC, 128], F16, kind="ExternalInput").ap()
    wk = nc.dram_tensor("wk", [128, DC, 128], F16, kind="ExternalInput").ap()
    wv = nc.dram_tensor("wv", [128, DC, 128], F16, kind="ExternalInput").ap()
    wo = nc.dram_tensor("wo", [128, D], F16, kind="ExternalInput").ap()
    pb = nc.dram_tensor(
        "pb", [QN, KGN, 128, KTG, HPC, QTS], F16, kind="ExternalInput"
    ).ap()
    mk = nc.dram_tensor("mk", [QN, 128, B, KN, QTS], U8, kind="ExternalInput").ap()
    ident = nc.dram_tensor("ident", [128, 128], F16, kind="ExternalInput").ap()
    identr = nc.dram_tensor("identr", [128, 128], F32R, kind="ExternalInput").ap()
    indh = nc.dram_tensor("indh", [1, 256], F16, kind="ExternalInput").ap()
    out = nc.dram_tensor("out", [B, L, D], F32, kind="ExternalOutput").ap()

    with tile.TileContext(nc) as tc:
        with (
            tc.tile_pool(name="const", bufs=1) as constp,
            tc.tile_pool(name="hq", bufs=2) as hqp,
            tc.tile_pool(name="hk", bufs=2) as hkp,
            tc.tile_pool(name="hv", bufs=2) as hvp,
            tc.tile_pool(name="stage", bufs=3) as stagep,
            tc.tile_pool(name="pbp", bufs=2) as pbp,
            tc.tile_pool(name="mask", bufs=2) as mkp,
            tc.tile_pool(name="ma", bufs=24) as map_,
            tc.tile_pool(name="pt", bufs=10) as ptp,
            tc.tile_pool(name="ctxn", bufs=2) as ctxnp,
            tc.tile_pool(name="rc", bufs=2) as rcp,
            tc.tile_pool(name="outb", bufs=3) as outp,
            tc.tile_pool(name="psum", bufs=8, space=bass.MemorySpace.PSUM) as psp,
        ):
            # ---- constants ----
            ident_t = constp.tile([128, 128], F16, tag="ident")
            nc.sync.dma_start(ident_t[:], ident[:])
            identr_t = constp.tile([128, 128], F32R, tag="identr")
            nc.sync.dma_start(identr_t[:], identr[:])
            indh_t = constp.tile([1, 256], F16, tag="indh")
            nc.sync.dma_start(indh_t[:], indh[:])
            wq_t = constp.tile([128, DC, 128], F16, tag="wq")
            nc.sync.dma_start(wq_t[:], wq[:])
            wk_t = constp.tile([128, DC, 128], F16, tag="wk")
            nc.sync.dma_start(wk_t[:], wk[:])
            wv_t = constp.tile([128, DC, 128], F16, tag="wv")
            nc.sync.dma_start(wv_t[:], wv[:])
            wo_t = constp.tile([128, D], F16, tag="wo")
            nc.sync.dma_start(wo_t[:], wo[:])

            # ---- prologue: projections ----
            hq_sb, hk_sb, hv_sb = {}, {}, {}
            for b in range(B):
                hq_sb[b] = hqp.tile([128, L], F16, tag="hq", name=f"hq_sb{b}")
                hq_ps = [
                    psp.tile([128, QTS], F32, tag="bank", name=f"hq_ps{b}_{i}")
                    for i in range(QN)
                ]
                for dc in range(DC):
                    qc = stagep.tile([128, L], F16, tag="stage")
                    nc.sync.dma_start(qc[:], qT[b, dc])
                    for qt in range(QN):
                        nc.tensor.matmul(
                            hq_ps[qt][:],
                            wq_t[:, dc, :],
                            qc[:, qt * QTS : (qt + 1) * QTS],
                            start=(dc == 0),
                            stop=(dc == DC - 1),
                        )
                for qt in range(QN):
                    nc.vector.tensor_copy(
                        hq_sb[b][:, qt * QTS : (qt + 1) * QTS], hq_ps[qt][:]
                    )

                hk_sb[b] = hkp.tile([128, L], F16, tag="hk", name=f"hk_sb{b}")
                hvT = stagep.tile([128, L], F32R, tag="stage")
                hk_ps = [
                    psp.tile([128, QTS], F32, tag="bank", name=f"hk_ps{b}_{i}")
                    for i in range(QN)
                ]
                hv_ps = [
                    psp.tile([128, QTS], F32, tag="bank", name=f"hv_ps{b}_{i}")
                    for i in range(QN)
                ]
                for dc in range(DC):
                    kc = stagep.tile([128, L], F16, tag="stage")
                    nc.sync.dma_start(kc[:], kvT[b, dc])
                    for qt in range(QN):
                        nc.tensor.matmul(
                            hk_ps[qt][:],
                            wk_t[:, dc, :],
                            kc[:, qt * QTS : (qt + 1) * QTS],
                            start=(dc == 0),
                            stop=(dc == DC - 1),
                        )
                        nc.tensor.matmul(
                            hv_ps[qt][:],
                            wv_t[:, dc, :],
                            kc[:, qt * QTS : (qt + 1) * QTS],
                            start=(dc == 0),
                            stop=(dc == DC - 1),
                        )
                for qt in range(QN):
                    nc.vector.tensor_copy(
                        hk_sb[b][:, qt * QTS : (qt + 1) * QTS], hk_ps[qt][:]
                    )
                    nc.vector.tensor_copy(
                        hvT[:, qt * QTS : (qt + 1) * QTS], hv_ps[qt][:]
                    )

                # hv_aug: transpose hvT per k-tile; ones cols prefilled
                hv_sb[b] = hvp.tile([128, KN * HVW], F16, tag="hv", name=f"hv_sb{b}")
                nc.gpsimd.memset(hv_sb[b][:].bitcast(mybir.dt.uint16), 0x3C00)
                for kt in range(KN):
                    tp = psp.tile([128, 128], F32R, tag="bank")
                    nc.tensor.transpose(
                        tp[:], hvT[:, kt * KP : (kt + 1) * KP], identr_t[:]
                    )
                    o = kt * HVW
                    nc.vector.tensor_copy(hv_sb[b][:, o : o + DH], tp[:, 0:DH])
                    nc.vector.tensor_copy(
                        hv_sb[b][:, o + DH + 1 : o + 2 * DH + 1], tp[:, DH:128]
                    )

            # ---- main loop ----
            for qt in range(QN):
                mk_t = {}
                for b in range(B):
                    mk_t[b] = mkp.tile(
                        [128, KN, QTS], U8, tag="mask", name=f"mk_t{b}_{qt}"
                    )
                    nc.scalar.dma_start(mk_t[b][:], mk[qt, :, b])
                ctx_ps = {}
                for b in range(B):
                    for h in range(HPC):
                        ctx_ps[(b, h)] = psp.tile(
                            [DH + 1, QTS], F32, tag="bank", name=f"ctx_ps{b}_{h}_{qt}"
                        )
                pending_pv = []
                for kg in range(KGN):
                    pb_t = pbp.tile([128, KTG, HPC, QTS], F16, tag="pb")
                    nc.scalar.dma_start(pb_t[:], pb[qt, kg])
                    # bias+mask fused elementwise, hoisted for the whole DMA
                    # group; split between DVE (b=0) and GpSimd (b=1)
                    cmb_g = {}
                    for ki in range(KTG):
                        kt = kg * KTG + ki
                        for b in range(B):
                            eng = nc.vector
                            for h in range(HPC):
                                c = map_.tile(
                                    [128, QTS], F16, tag="ma", name=f"cmb{b}_{h}_{kt}"
                                )
                                eng.scalar_tensor_tensor(
                                    c[:],
                                    mk_t[b][:, kt, :],
                                    -MM_NEG,
                                    pb_t[:, ki, h, :],
                                    mybir.AluOpType.mult,
                                    mybir.AluOpType.add,
                                )
                                cmb_g[(b, h, kt)] = c
                    for ki in range(KTG):
                        kt = kg * KTG + ki
                        cmb = {
                            (b, h): cmb_g[(b, h, kt)]
                            for b in range(B)
                            for h in range(HPC)
                        }
                        sc = {}
                        # identity matmuls: same stationary, batched
                        for b in range(B):
                            for h in range(HPC):
                                sc[(b, h)] = psp.tile(
                                    [128, QTS], F32, tag="bank", name=f"sc{b}_{h}_{kt}"
                                )
                                nc.tensor.matmul(
                                    sc[(b, h)][:],
                                    ident_t[:],
                                    cmb[(b, h)][:],
                                    start=True,
                                    stop=False,
                                )
                        # QK: h0 rows 0-63 / h1 rows 64-127 run concurrently
                        for b in range(B):
                            for h in range(HPC):
                                nc.tensor.matmul(
                                    sc[(b, h)][:],
                                    hk_sb[b][
                                        h * DH : (h + 1) * DH, kt * KP : (kt + 1) * KP
                                    ],
                                    hq_sb[b][
                                        h * DH : (h + 1) * DH,
                                        qt * QTS : (qt + 1) * QTS,
                                    ],
                                    start=False,
                                    stop=True,
                                )
                        new_pv = []
                        for b in range(B):
                            for h in range(HPC):
                                p_t = ptp.tile(
                                    [128, QTS], F16, tag="pt", name=f"p_t{b}_{h}_{kt}"
                                )
                                nc.scalar.activation(
                                    p_t[:],
                                    sc[(b, h)][:],
                                    mybir.ActivationFunctionType.Exp,
                                )
                                new_pv.append((b, h, kt, p_t))
                        # software pipeline: PV of previous k-tile (its exp is
                        # certainly done; PE is in-order)
                        for b, h, pkt, p_t in pending_pv:
                            o = pkt * HVW + h * (DH + 1)
                            nc.tensor.matmul(
                                ctx_ps[(b, h)][:],
                                hv_sb[b][:, o : o + DH + 1],
                                p_t[:],
                                start=(pkt == 0),
                                stop=(pkt == KN - 1),
                            )
                        pending_pv = new_pv
                for b, h, pkt, p_t in pending_pv:
                    o = pkt * HVW + h * (DH + 1)
                    nc.tensor.matmul(
                        ctx_ps[(b, h)][:],
                        hv_sb[b][:, o : o + DH + 1],
                        p_t[:],
                        start=(pkt == 0),
                        stop=(pkt == KN - 1),
                    )
                # normalize + output projection
                for b in range(B):
                    ctxn = ctxnp.tile([128, QTS], F16, tag="ctxn")
                    bcw = psp.tile([128, QTS], F32, tag="bank", name=f"bcw{b}")
                    bc = bcw[:]
                    for h in range(HPC):
                        dsb = rcp.tile([1, QTS], F32, tag="dsb", name=f"dsb{b}_{h}")
                        nc.vector.tensor_copy(dsb[:], ctx_ps[(b, h)][DH : DH + 1, :])
                        rcf = rcp.tile([1, QTS], F32, tag="rcf", name=f"rcf{b}_{h}")
                        nc.vector.reciprocal_approx_fast(rcf[:], dsb[:])
                        rcr = rcp.tile([1, QTS], F16, tag="rcr", name=f"rcr{b}_{h}")
                        nc.vector.tensor_copy(rcr[:], rcf[:])
                        nc.tensor.matmul(
                            bc,
                            indh_t[:, h * 128 : (h + 1) * 128],
                            rcr[:],
                            start=(h == 0),
                            stop=(h == HPC - 1),
                        )
                    bc_sb = rcp.tile([128, QTS], F32, tag="bcsb", name=f"bc_sb{b}")
                    nc.vector.tensor_copy(bc_sb[:], bc)
                    for h in range(HPC):
                        nc.vector.tensor_tensor(
                            ctxn[h * DH : (h + 1) * DH, :],
                            ctx_ps[(b, h)][0:DH, :],
                            bc_sb[h * DH : (h + 1) * DH, :],
                            mybir.AluOpType.mult,
                        )
                    for qs in range(QN):
                        ob = outp.tile([128, D], F32, tag="outb")
                        for oh in range(2):
                            op_ps = psp.tile(
                                [128, QTS], F32, tag="bank", name=f"op{b}_{qs}_{oh}"
                            )
                            nc.tensor.matmul(
                                op_ps[:],
                                ctxn[:, qs * 128 : (qs + 1) * 128],
                                wo_t[:, oh * QTS : (oh + 1) * QTS],
                                start=True,
                                stop=True,
                            )
                            if oh == 0:
                                nc.vector.tensor_copy(
                                    ob[:, oh * QTS : (oh + 1) * QTS], op_ps[:]
                                )
                            else:
                                nc.scalar.copy(
                                    ob[:, oh * QTS : (oh + 1) * QTS], op_ps[:]
                                )
                        r0 = qt * QTS + qs * 128
                        nc.sync.dma_start(out[b, r0 : r0 + 128, :], ob[:])

    nc.compile()
    _CACHE["nc"] = nc
    return nc


def _prep_core(core, query, key_value, mask, position_bias, Wq, Wk, Wv, Wo, shared):
    """Per-core input map. `shared` holds core-independent packed arrays."""
    h0 = core * HPC
    rows = slice(h0 * DH, (h0 + HPC) * DH)

    def packw(w, scale=1.0):
        return np.ascontiguousarray(
            (w[rows].T * scale).reshape(DC, 128, 128).transpose(1, 0, 2)
        ).astype(np.float16)

    pbc = position_bias[h0 : h0 + HPC]  # [2, q, k]
    # -> [qt, kg, kp, ki, h, qf]
    pbp = np.ascontiguousarray(
        pbc.reshape(HPC, QN, QTS, KGN, KTG, 128).transpose(1, 3, 5, 4, 0, 2)
    ).astype(np.float16)
    return {
        "qT": shared["qT"],
        "kvT": shared["kvT"],
        "mk": shared["mk"],
        "ident": shared["ident"],
        "identr": shared["identr"],
        "indh": shared["indh"],
        "wq": packw(Wq, 1.0 / np.sqrt(DH)),
        "wk": packw(Wk),
        "wv": packw(Wv),
        "wo": np.ascontiguousarray(Wo[:, rows].T).astype(np.float16),
        "pb": pbp,
    }


def _prep_shared(query, key_value, mask):
    qTp = np.ascontiguousarray(
        query.reshape(B, L, DC, 128).transpose(0, 2, 3, 1)
    ).astype(np.float16)
    kvTp = np.ascontiguousarray(
        key_value.reshape(B, L, DC, 128).transpose(0, 2, 3, 1)
    ).astype(np.float16)
    mku = (~np.asarray(mask, dtype=bool)).astype(np.uint8)  # 1 = masked out
    # [b, q, k] -> [qt, kp, b, kt, qf]
    mkp = np.ascontiguousarray(
        mku.reshape(B, QN, QTS, KN, 128).transpose(1, 4, 0, 3, 2)
    )
    indh = np.concatenate(
        [
            np.where(np.arange(128) < 64, 1.0, 0.0),
            np.where(np.arange(128) >= 64, 1.0, 0.0),
        ]
    ).astype(np.float16)[None, :]
    return {
        "qT": qTp,
        "kvT": kvTp,
        "mk": mkp,
        "ident": np.eye(128, dtype=np.float16),
        "identr": np.eye(128, dtype=np.float32),
        "indh": np.ascontiguousarray(indh),
    }


def kernel(query, key_value, mask, position_bias, Wq, Wk, Wv, Wo, _trace=False):
    query = np.asarray(query, dtype=np.float32)
    key_value = np.asarray(key_value, dtype=np.float32)
    mask = np.asarray(mask)
    position_bias = np.asarray(position_bias, dtype=np.float32)
    Wq = np.asarray(Wq, dtype=np.float32)
    Wk = np.asarray(Wk, dtype=np.float32)
    Wv = np.asarray(Wv, dtype=np.float32)
    Wo = np.asarray(Wo, dtype=np.float32)

    nc = _build()
    shared = _prep_shared(query, key_value, mask)
    in_maps = [
        _prep_core(c, query, key_value, mask, position_bias, Wq, Wk, Wv, Wo, shared)
        for c in range(N_CORES)
    ]
    res = run_bass_kernel_spmd(nc, in_maps, list(range(N_CORES)), trace=_trace)
    _CACHE["last_result"] = res
    acc = res.results[0]["out"].astype(np.float64)
    for c in range(1, N_CORES):
        acc += res.results[c]["out"]
    return acc.astype(np.float32)
